# revision 13
# baseline (speedup 1.0000x reference)
"""CRF NLL kernel for Trainium2 (8 NeuronCores, batch-sharded).

Math (validated in numpy): forward algorithm in exp-space with a constant
per-step normalizer C, split bidirectionally (fwd from s=0, bwd from s=1023)
to halve the serial recursion depth. Both chains run fused in one
128-partition pipeline: partitions 0-63 carry the fwd state, 64-127 the bwd
state; one block-diagonal 128x128 stationary matmul + one [128,64] vector
multiply per slot (511 slots).

Score side: the emission gather reads the SAME transposed emission chunks as
the recursion (single HBM pass): per chunk, the matching tag values are
broadcast across partitions with a K=1 ones-matmul on TensorE, then one fused
DVE op computes (tag_bcast == iota_partition) * em with per-partition
accumulation. Transition/start/end terms are dot products of host-side
integer tag bincounts with the parameter tensors (device does all float math
touching parameters).

Output: per-core partial sums [1,8]; host combines and takes the mean.
"""

import numpy as np

S, B, T, NCORES = 1024, 512, 64, 8
BC = B // NCORES          # 64 batch per core
NSLOT = (S - 2) // 2      # 511 recursion slots per chain
CH_E = 7                  # slots per e-chunk (73 * 7 = 511)
NCH_E = NSLOT // CH_E     # 73
CW = CH_E * T             # 448 chunk width
CNORM = 4.66              # constant per-step normalizer (log-space)

_COMPILED = {}


def _build_program(repeat=1):
    import contextlib
    from contextlib import ExitStack

    import concourse.bacc as bacc
    import concourse.tile as tile
    import concourse.mybir as mybir

    f32 = mybir.dt.float32
    bf16 = mybir.dt.bfloat16
    Exp = mybir.ActivationFunctionType.Exp
    Log = mybir.ActivationFunctionType.Ln
    mult = mybir.AluOpType.mult
    add = mybir.AluOpType.add
    is_equal = mybir.AluOpType.is_equal
    AX = mybir.AxisListType

    nc = bacc.Bacc(
        "TRN2",
        target_bir_lowering=False,
        debug=False,
        enable_asserts=False,
        num_devices=NCORES,
    )

    def din(name, shape, dt=f32):
        return nc.dram_tensor(name, shape, dt, kind="ExternalInput").ap()

    em_stack = din("em_stack", [NCH_E, 128, CW])      # transposed slot-stacked em
    em_init = din("em_init", [128, BC])               # em[0].T / em[1023].T
    tags_rows = din("tags_rows", [1, 65536], bf16)    # tag value per em_stack col
    iota_col = din("iota_col", [128, 1])              # 0..63,0..63 per partition
    ones64 = din("ones64", [1, T], bf16)              # K=1 broadcast weights
    trans2 = din("trans2", [128, T])                  # [trans ; trans^T] stacked
    cpair = din("cpair", [T, T])                      # pair bincount (f32)
    cse = din("cse", [128, 1])                        # [count_start ; count_end]
    pse = din("pse", [128, 1])                        # [start ; end] transitions
    out_part = nc.dram_tensor("out_part", [1, 8], f32, kind="ExternalOutput").ap()

    with tile.TileContext(nc) as tc, ExitStack() as ctx:
        const = ctx.enter_context(tc.tile_pool(name="const", bufs=1))
        eraw_p = ctx.enter_context(tc.tile_pool(name="eraw", bufs=4))
        eexp_p = ctx.enter_context(tc.tile_pool(name="eexp", bufs=3))
        alpha_p = ctx.enter_context(tc.tile_pool(name="alpha", bufs=3))
        scr_p = ctx.enter_context(tc.tile_pool(name="scr", bufs=2))
        small_p = ctx.enter_context(tc.tile_pool(name="small", bufs=1))
        psum_p = ctx.enter_context(tc.tile_pool(name="psum", bufs=2, space="PSUM"))
        psumf_p = ctx.enter_context(tc.tile_pool(name="psumf", bufs=1, space="PSUM"))
        psumb_p = ctx.enter_context(tc.tile_pool(name="psumb", bufs=2, space="PSUM"))

        # ---- constants into SBUF
        iota_sb = const.tile([128, 1], f32)
        nc.sync.dma_start(iota_sb[:], iota_col)
        ones_sb = const.tile([1, T], bf16)
        nc.sync.dma_start(ones_sb[:], ones64)
        tagr_sb = const.tile([1, 65536], bf16)
        nc.sync.dma_start(tagr_sb[:], tags_rows)
        tt_sb = const.tile([128, T], f32)
        nc.sync.dma_start(tt_sb[:], trans2)
        cpair_sb = const.tile([T, T], f32)
        nc.sync.dma_start(cpair_sb[:], cpair)
        cse_sb = const.tile([128, 1], f32)
        nc.sync.dma_start(cse_sb[:], cse)
        pse_sb = const.tile([128, 1], f32)
        nc.sync.dma_start(pse_sb[:], pse)
        eminit_sb = const.tile([128, BC], f32)
        nc.sync.dma_start(eminit_sb[:], em_init)

        # ---- stationary weights: W = [[expT, 0], [0, expT^T]], W2 = [[0],[expT^T]]
        W = const.tile([128, 128], f32)
        nc.vector.memset(W[:], 0.0)
        nc.scalar.activation(W[0:64, 0:64], tt_sb[0:64, :], Exp)
        nc.scalar.activation(W[64:128, 64:128], tt_sb[64:128, :], Exp)
        W2 = const.tile([128, T], f32)
        nc.vector.memset(W2[:], 0.0)
        nc.scalar.activation(W2[64:128, :], tt_sb[64:128, :], Exp)

        # ---- init state: alpha0 = exp(em_init + [start; end - C])
        negc_col = const.tile([128, 1], f32)
        nc.vector.memset(negc_col[:], -CNORM)
        bias_col = const.tile([128, 1], f32)
        nc.vector.tensor_copy(bias_col[0:64, :], pse_sb[0:64, :])
        nc.vector.tensor_scalar_add(bias_col[64:128, :], pse_sb[64:128, :], -CNORM)
        alpha = alpha_p.tile([128, BC], f32)
        nc.scalar.activation(alpha[:], eminit_sb[:], Exp, bias=bias_col[:, 0:1])

        # ---- emission-gather accumulator: two columns per chunk (+1 for em_init)
        emit_acc = const.tile([128, 2 * NCH_E + 1], f32)

        def emit_part(src_ap, fw_cols, bw_cols, width, acc_idx):
            """sum_j src[j,c]*[tag(c)==j] accumulated per partition, one DVE op.

            Tag values for the columns are broadcast down the partitions by a
            K=1 matmul with a ones stationary (TensorE is idle-heavy here, the
            broadcasts ride in the chain's latency gaps). Split into sub-chunk
            parts so each DVE op fits the per-slot dependency-latency window.
            """
            bc_ps = psumb_p.tile([128, width], f32)
            nc.tensor.matmul(bc_ps[0:64, :], ones_sb[:],
                             tagr_sb[:, fw_cols:fw_cols + width],
                             start=True, stop=True)
            nc.tensor.matmul(bc_ps[64:128, :], ones_sb[:],
                             tagr_sb[:, bw_cols:bw_cols + width],
                             start=True, stop=True, tile_position=(0, 64))
            scratch = scr_p.tile([128, width], f32)
            nc.vector.scalar_tensor_tensor(
                scratch[:],
                bc_ps[:],
                iota_sb[:, 0:1],
                src_ap,
                op0=is_equal,
                op1=mult,
                accum_out=emit_acc[:, acc_idx:acc_idx + 1],
            )

        # ---- bidirectional recursion chain; per chunk: DMA -> exp (ACT),
        # 7 matmul+mult slots, then the chunk's emission gather (DVE) which
        # fills the chain's dependency-latency gaps.
        rep_ctx = tc.For_i(0, repeat, 1) if repeat > 1 else contextlib.nullcontext()
        ctx.enter_context(rep_ctx)
        if repeat > 1:
            alpha = alpha_p.tile([128, BC], f32)
            nc.scalar.activation(alpha[:], eminit_sb[:], Exp, bias=bias_col[:, 0:1])

        for g in range(NCH_E):
            raw = eraw_p.tile([128, CW], f32)
            # Alternate chunks across the two HWDGE queues (SP / Activation)
            # so each queue sustains ~25 GB/s instead of ~50 on one — a late
            # chunk would stall the recursion through the exp->mult dep.
            dma_eng = nc.sync if g % 2 == 0 else nc.scalar
            dma_eng.dma_start(raw[:], em_stack[g])
            e = eexp_p.tile([128, CW], f32)
            nc.scalar.activation(e[:], raw[:], Exp, bias=negc_col[:, 0:1])
            half = CW // 2  # 224
            for k in range(CH_E):
                gamma = psum_p.tile([128, BC], f32)
                nc.tensor.matmul(gamma[:], W[:], alpha[:], start=True, stop=True)
                alpha = alpha_p.tile([128, BC], f32)
                nc.vector.tensor_mul(alpha[:], gamma[:], e[:, k * T:(k + 1) * T])
                if k == 3:
                    emit_part(raw[:, 0:half], g * 2 * CW, g * 2 * CW + CW,
                              half, 2 * g)
                elif k == 6:
                    emit_part(raw[:, half:CW], g * 2 * CW + half,
                              g * 2 * CW + CW + half, half, 2 * g + 1)
        emit_part(eminit_sb[:], NCH_E * 2 * CW, NCH_E * 2 * CW + BC, BC,
                  2 * NCH_E)

        # ---- join: Z_b = sum_i alpha_fwd[i,b] * (expT @ bt)[i,b]
        gfin = psumf_p.tile([64, BC], f32)
        nc.tensor.matmul(gfin[:], W2[:], alpha[:], start=True, stop=True)
        zprod = small_p.tile([64, BC], f32)
        nc.vector.tensor_mul(zprod[:], gfin[:], alpha[0:64, :])
        zcol = small_p.tile([1, BC], f32)
        nc.gpsimd.tensor_reduce(zcol[:], zprod[:], axis=AX.C, op=add)
        logz = small_p.tile([1, BC], f32)
        nc.scalar.activation(logz[:], zcol[:], Log)
        logz_sum = small_p.tile([1, 1], f32)
        nc.vector.tensor_reduce(logz_sum[:], logz[:], axis=AX.X, op=add)

        # ---- score dot products
        emit_col = small_p.tile([128, 1], f32)
        nc.vector.tensor_reduce(emit_col[:], emit_acc[:], axis=AX.X, op=add)
        emit_sum = small_p.tile([1, 1], f32)
        nc.gpsimd.tensor_reduce(emit_sum[:], emit_col[:], axis=AX.C, op=add)

        tscr = small_p.tile([T, T], f32)
        td_col = small_p.tile([T, 1], f32)
        nc.vector.scalar_tensor_tensor(
            tscr[:], cpair_sb[:], 1.0, tt_sb[0:64, :],
            op0=mult, op1=mult, accum_out=td_col[:],
        )
        trans_dot = small_p.tile([1, 1], f32)
        nc.gpsimd.tensor_reduce(trans_dot[:], td_col[:], axis=AX.C, op=add)

        se_col = small_p.tile([128, 1], f32)
        nc.vector.tensor_mul(se_col[:], cse_sb[:], pse_sb[:])
        se_sum = small_p.tile([1, 1], f32)
        nc.gpsimd.tensor_reduce(se_sum[:], se_col[:], axis=AX.C, op=add)

        # ---- ship partials
        nc.sync.dma_start(out_part[0:1, 0:1], logz_sum[:])
        nc.sync.dma_start(out_part[0:1, 1:2], emit_sum[:])
        nc.sync.dma_start(out_part[0:1, 2:3], trans_dot[:])
        nc.sync.dma_start(out_part[0:1, 3:4], se_sum[:])

    nc.compile()
    return nc


def _get_compiled(repeat=1):
    if repeat not in _COMPILED:
        _COMPILED[repeat] = _build_program(repeat)
    return _COMPILED[repeat]


def _prep_core(em_c, tags_c, trans, start, end, iota_arr=None):
    """Build the per-core input map (numpy only; index/layout prep + bincounts)."""
    import ml_dtypes

    emT = np.ascontiguousarray(em_c.transpose(0, 2, 1))      # [S, T, BC]
    stack = np.empty((NSLOT, 128, BC), np.float32)
    stack[:, :64, :] = emT[1:1 + NSLOT]                       # fwd: em[1..511]
    stack[:, 64:, :] = emT[S - 2:S - 2 - NSLOT:-1]            # bwd: em[1022..512]
    em_stack = np.ascontiguousarray(
        stack.reshape(NCH_E, CH_E, 128, BC).transpose(0, 2, 1, 3).reshape(NCH_E, 128, CW)
    )
    em_init = np.concatenate([emT[0], emT[S - 1]], axis=0).astype(np.float32)

    # tag value per em_stack column: chunk g = [fwd slots g*7+k -> em step
    # g*7+k+1] then [bwd slots -> em step 1022-(g*7+k)]; tail 128 cols for
    # em_init (step 0 then step 1023)
    tr_rows = np.empty(65536, np.float32)
    for g in range(NCH_E):
        base = g * 2 * CW
        tr_rows[base:base + CW] = tags_c[1 + 7 * g:8 + 7 * g].reshape(-1)
        tr_rows[base + CW:base + 2 * CW] = \
            tags_c[1016 - 7 * g:1023 - 7 * g][::-1].reshape(-1)
    tail = NCH_E * 2 * CW
    tr_rows[tail:tail + BC] = tags_c[0]
    tr_rows[tail + BC:tail + 2 * BC] = tags_c[-1]
    tr_rows[tail + 2 * BC:] = 0.0
    tags_rows = tr_rows.reshape(1, 65536).astype(ml_dtypes.bfloat16)

    cpair = np.bincount(
        (tags_c[:-1].astype(np.int64) * T + tags_c[1:]).reshape(-1), minlength=T * T
    ).reshape(T, T).astype(np.float32)
    cs = np.bincount(tags_c[0], minlength=T).astype(np.float32)
    ce = np.bincount(tags_c[-1], minlength=T).astype(np.float32)
    return {
        "em_stack": em_stack,
        "em_init": em_init,
        "tags_rows": tags_rows,
        "iota_col": np.concatenate([np.arange(T), np.arange(T)]).reshape(128, 1).astype(np.float32),
        "ones64": np.ones((1, T), ml_dtypes.bfloat16),
        "trans2": np.concatenate([trans, trans.T], axis=0).astype(np.float32),
        "cpair": cpair,
        "cse": np.concatenate([cs, ce]).reshape(128, 1).astype(np.float32),
        "pse": np.concatenate([start, end]).reshape(128, 1).astype(np.float32),
    }


def kernel(emissions, tags, mask, transitions, start_transitions, end_transitions,
           _trace=False):
    from concourse.bass_utils import run_bass_kernel_spmd

    em = np.asarray(emissions, np.float32)
    tg = np.asarray(tags)
    tr = np.asarray(transitions, np.float32)
    st = np.asarray(start_transitions, np.float32)
    en = np.asarray(end_transitions, np.float32)
    # mask is all-ones in this problem setup; sequence lengths are full.

    in_maps = []
    for c in range(NCORES):
        sl = slice(c * BC, (c + 1) * BC)
        in_maps.append(_prep_core(
            np.ascontiguousarray(em[:, sl, :]),
            np.ascontiguousarray(tg[:, sl]).astype(np.int64),
            tr, st, en,
        ))

    nc = _get_compiled()
    res = run_bass_kernel_spmd(nc, in_maps, core_ids=list(range(NCORES)),
                               trace=_trace)
    total = 0.0
    for c in range(NCORES):
        p = res.results[c]["out_part"].reshape(-1).astype(np.float64)
        logz_sum, emit_sum, trans_dot, se_sum = p[0], p[1], p[2], p[3]
        logz_sum += BC * (S - 1) * CNORM
        total += logz_sum - (emit_sum + trans_dot + se_sum)
    out = np.float32(total / B)
    if _trace:
        return out, res
    return out


# revision 17
# speedup vs baseline: 3.5781x; 3.5781x over previous
"""CRF NLL kernel for Trainium2 (8 NeuronCores, batch-sharded).

Math (validated in numpy): forward algorithm in exp-space with a constant
per-step normalizer C, split bidirectionally (fwd from s=0, bwd from s=1023)
to halve the serial recursion depth. Both chains run fused in one
128-partition pipeline: partitions 0-63 carry the fwd state, 64-127 the bwd
state; one block-diagonal 128x128 stationary matmul + one [128,64] vector
multiply per slot (511 slots).

Score side: the emission gather reads the SAME transposed emission chunks as
the recursion (single HBM pass): per chunk, the matching tag values are
broadcast across partitions with a K=1 ones-matmul on TensorE, then one fused
DVE op computes (tag_bcast == iota_partition) * em with per-partition
accumulation. Transition/start/end terms are dot products of host-side
integer tag bincounts with the parameter tensors (device does all float math
touching parameters).

Output: per-core partial sums [1,8]; host combines and takes the mean.
"""

import numpy as np

S, B, T, NCORES = 1024, 512, 64, 8
BC = B // NCORES          # 64 batch per core
NSLOT = (S - 2) // 2      # 511 recursion slots per chain
CH_E = 7                  # slots per e-chunk (73 * 7 = 511)
NCH_E = NSLOT // CH_E     # 73
CW = CH_E * T             # 448 chunk width
CNORM = 4.66              # constant per-step normalizer (log-space)

_COMPILED = {}


def _build_program(repeat=1):
    import contextlib
    from contextlib import ExitStack

    import concourse.bacc as bacc
    import concourse.tile as tile
    import concourse.mybir as mybir

    f32 = mybir.dt.float32
    bf16 = mybir.dt.bfloat16
    Exp = mybir.ActivationFunctionType.Exp
    Log = mybir.ActivationFunctionType.Ln
    mult = mybir.AluOpType.mult
    add = mybir.AluOpType.add
    is_equal = mybir.AluOpType.is_equal
    AX = mybir.AxisListType

    nc = bacc.Bacc(
        "TRN2",
        target_bir_lowering=False,
        debug=False,
        enable_asserts=False,
        num_devices=NCORES,
    )

    def din(name, shape, dt=f32):
        return nc.dram_tensor(name, shape, dt, kind="ExternalInput").ap()

    em_stack = din("em_stack", [NCH_E, 128, CW])      # transposed slot-stacked em
    em_init = din("em_init", [128, BC])               # em[0].T / em[1023].T
    tags_rows = din("tags_rows", [1, 65536], bf16)    # tag value per em_stack col
    iota_col = din("iota_col", [128, 1])              # 0..63,0..63 per partition
    ones64 = din("ones64", [1, T], bf16)              # K=1 broadcast weights
    trans2 = din("trans2", [128, T])                  # [trans ; trans^T] stacked
    cpair = din("cpair", [T, T])                      # pair bincount (f32)
    cse = din("cse", [128, 1])                        # [count_start ; count_end]
    pse = din("pse", [128, 1])                        # [start ; end] transitions
    out_part = nc.dram_tensor("out_part", [1, 8], f32, kind="ExternalOutput").ap()

    with tile.TileContext(nc) as tc, ExitStack() as ctx:
        const = ctx.enter_context(tc.tile_pool(name="const", bufs=1))
        eraw_p = ctx.enter_context(tc.tile_pool(name="eraw", bufs=5))
        eexp_p = ctx.enter_context(tc.tile_pool(name="eexp", bufs=4))
        alpha_p = ctx.enter_context(tc.tile_pool(name="alpha", bufs=4))
        scr_p = ctx.enter_context(tc.tile_pool(name="scr", bufs=3))
        small_p = ctx.enter_context(tc.tile_pool(name="small", bufs=1))
        psum_p = ctx.enter_context(tc.tile_pool(name="psum", bufs=3, space="PSUM"))
        psumf_p = ctx.enter_context(tc.tile_pool(name="psumf", bufs=1, space="PSUM"))
        psumb_p = ctx.enter_context(tc.tile_pool(name="psumb", bufs=2, space="PSUM"))

        # ---- constants into SBUF
        iota_sb = const.tile([128, 1], f32)
        nc.sync.dma_start(iota_sb[:], iota_col)
        ones_sb = const.tile([1, T], bf16)
        nc.sync.dma_start(ones_sb[:], ones64)
        tagr_sb = const.tile([1, 65536], bf16)
        nc.sync.dma_start(tagr_sb[:], tags_rows)
        tt_sb = const.tile([128, T], f32)
        nc.sync.dma_start(tt_sb[:], trans2)
        cpair_sb = const.tile([T, T], f32)
        nc.sync.dma_start(cpair_sb[:], cpair)
        cse_sb = const.tile([128, 1], f32)
        nc.sync.dma_start(cse_sb[:], cse)
        pse_sb = const.tile([128, 1], f32)
        nc.sync.dma_start(pse_sb[:], pse)
        eminit_sb = const.tile([128, BC], f32)
        nc.sync.dma_start(eminit_sb[:], em_init)

        # ---- stationary weights: W = [[expT, 0], [0, expT^T]], W2 = [[0],[expT^T]]
        W = const.tile([128, 128], f32)
        nc.vector.memset(W[:], 0.0)
        nc.scalar.activation(W[0:64, 0:64], tt_sb[0:64, :], Exp)
        nc.scalar.activation(W[64:128, 64:128], tt_sb[64:128, :], Exp)
        W2 = const.tile([128, T], f32)
        nc.vector.memset(W2[:], 0.0)
        nc.scalar.activation(W2[64:128, :], tt_sb[64:128, :], Exp)

        # ---- init state: alpha0 = exp(em_init + [start; end - C])
        negc_col = const.tile([128, 1], f32)
        nc.vector.memset(negc_col[:], -CNORM)
        bias_col = const.tile([128, 1], f32)
        nc.vector.tensor_copy(bias_col[0:64, :], pse_sb[0:64, :])
        nc.vector.tensor_scalar_add(bias_col[64:128, :], pse_sb[64:128, :], -CNORM)
        alpha = alpha_p.tile([128, BC], f32)
        nc.scalar.activation(alpha[:], eminit_sb[:], Exp, bias=bias_col[:, 0:1])

        # ---- emission-gather accumulator: two columns per chunk (+1 for em_init)
        emit_acc = const.tile([128, 2 * NCH_E + 1], f32)

        def emit_part(src_ap, fw_cols, bw_cols, width, acc_idx):
            """sum_j src[j,c]*[tag(c)==j] accumulated per partition, one DVE op.

            Tag values for the columns are broadcast down the partitions by a
            K=1 matmul with a ones stationary (TensorE is idle-heavy here, the
            broadcasts ride in the chain's latency gaps). Split into sub-chunk
            parts so each DVE op fits the per-slot dependency-latency window.
            """
            bc_ps = psumb_p.tile([128, width], f32)
            nc.tensor.matmul(bc_ps[0:64, :], ones_sb[:],
                             tagr_sb[:, fw_cols:fw_cols + width],
                             start=True, stop=True)
            nc.tensor.matmul(bc_ps[64:128, :], ones_sb[:],
                             tagr_sb[:, bw_cols:bw_cols + width],
                             start=True, stop=True, tile_position=(0, 64))
            scratch = scr_p.tile([128, width], f32)
            nc.vector.scalar_tensor_tensor(
                scratch[:],
                bc_ps[:],
                iota_sb[:, 0:1],
                src_ap,
                op0=is_equal,
                op1=mult,
                accum_out=emit_acc[:, acc_idx:acc_idx + 1],
            )

        # ---- bidirectional recursion chain; per chunk: DMA -> exp (ACT),
        # 7 matmul+mult slots, then the chunk's emission gather (DVE) which
        # fills the chain's dependency-latency gaps.
        rep_ctx = tc.For_i(0, repeat, 1) if repeat > 1 else contextlib.nullcontext()
        ctx.enter_context(rep_ctx)
        if repeat > 1:
            alpha = alpha_p.tile([128, BC], f32)
            nc.scalar.activation(alpha[:], eminit_sb[:], Exp, bias=bias_col[:, 0:1])

        for g in range(NCH_E):
            raw = eraw_p.tile([128, CW], f32)
            # Alternate chunks across the two HWDGE queues (SP / Activation)
            # so each queue sustains ~25 GB/s instead of ~50 on one — a late
            # chunk would stall the recursion through the exp->mult dep.
            dma_eng = nc.sync if g % 2 == 0 else nc.scalar
            dma_eng.dma_start(raw[:], em_stack[g])
            e = eexp_p.tile([128, CW], f32)
            nc.scalar.activation(e[:], raw[:], Exp, bias=negc_col[:, 0:1])
            half = CW // 2  # 224
            for k in range(CH_E):
                gamma = psum_p.tile([128, BC], f32)
                nc.tensor.matmul(gamma[:], W[:], alpha[:], start=True, stop=True)
                alpha = alpha_p.tile([128, BC], f32)
                nc.vector.tensor_mul(alpha[:], gamma[:], e[:, k * T:(k + 1) * T])
                if k == 3:
                    emit_part(raw[:, 0:half], g * 2 * CW, g * 2 * CW + CW,
                              half, 2 * g)
                elif k == 6:
                    emit_part(raw[:, half:CW], g * 2 * CW + half,
                              g * 2 * CW + CW + half, half, 2 * g + 1)
        emit_part(eminit_sb[:], NCH_E * 2 * CW, NCH_E * 2 * CW + BC, BC,
                  2 * NCH_E)

        # All partition-axis reductions below go through ones-vector matmuls
        # on the (idle) TensorE — gpsimd.tensor_reduce(axis=C) is a slow
        # software loop and would sit serially on the post-chain tail.
        ones_col = const.tile([128, 1], f32)
        nc.vector.memset(ones_col[:], 1.0)

        # ---- join: Z_b = sum_i alpha_fwd[i,b] * (expT @ bt)[i,b]
        gfin = psumf_p.tile([64, BC], f32)
        nc.tensor.matmul(gfin[:], W2[:], alpha[:], start=True, stop=True)
        zprod = small_p.tile([64, BC], f32)
        nc.vector.tensor_mul(zprod[:], gfin[:], alpha[0:64, :])
        zps = psumf_p.tile([1, BC], f32)
        nc.tensor.matmul(zps[:], ones_col[0:64, 0:1], zprod[:], start=True, stop=True)
        logz = small_p.tile([1, BC], f32)
        nc.scalar.activation(logz[:], zps[:], Log)
        logz_sum = small_p.tile([1, 1], f32)
        nc.vector.tensor_reduce(logz_sum[:], logz[:], axis=AX.X, op=add)

        # ---- score dot products, stacked into one [128,3] tile so a single
        # ones-matmul reduces all three partition-wise sums at once
        stacked = small_p.tile([128, 3], f32)
        nc.vector.memset(stacked[:], 0.0)
        nc.vector.tensor_reduce(stacked[:, 0:1], emit_acc[:], axis=AX.X, op=add)
        tscr = small_p.tile([T, T], f32)
        nc.vector.scalar_tensor_tensor(
            tscr[:], cpair_sb[:], 1.0, tt_sb[0:64, :],
            op0=mult, op1=mult, accum_out=stacked[0:64, 1:2],
        )
        nc.vector.tensor_mul(stacked[:, 2:3], cse_sb[:], pse_sb[:])
        sums_ps = psumf_p.tile([1, 3], f32)
        nc.tensor.matmul(sums_ps[:], ones_col[:], stacked[:], start=True, stop=True)
        sums_sb = small_p.tile([1, 3], f32)
        nc.vector.tensor_copy(sums_sb[:], sums_ps[:])

        # ---- ship partials
        nc.sync.dma_start(out_part[0:1, 0:1], logz_sum[:])
        nc.sync.dma_start(out_part[0:1, 1:4], sums_sb[:])

    nc.compile()
    return nc


def _get_compiled(repeat=1):
    if repeat not in _COMPILED:
        _COMPILED[repeat] = _build_program(repeat)
    return _COMPILED[repeat]


def _prep_core(em_c, tags_c, trans, start, end, iota_arr=None):
    """Build the per-core input map (numpy only; index/layout prep + bincounts)."""
    import ml_dtypes

    emT = np.ascontiguousarray(em_c.transpose(0, 2, 1))      # [S, T, BC]
    stack = np.empty((NSLOT, 128, BC), np.float32)
    stack[:, :64, :] = emT[1:1 + NSLOT]                       # fwd: em[1..511]
    stack[:, 64:, :] = emT[S - 2:S - 2 - NSLOT:-1]            # bwd: em[1022..512]
    em_stack = np.ascontiguousarray(
        stack.reshape(NCH_E, CH_E, 128, BC).transpose(0, 2, 1, 3).reshape(NCH_E, 128, CW)
    )
    em_init = np.concatenate([emT[0], emT[S - 1]], axis=0).astype(np.float32)

    # tag value per em_stack column: chunk g = [fwd slots g*7+k -> em step
    # g*7+k+1] then [bwd slots -> em step 1022-(g*7+k)]; tail 128 cols for
    # em_init (step 0 then step 1023)
    tr_rows = np.empty(65536, np.float32)
    for g in range(NCH_E):
        base = g * 2 * CW
        tr_rows[base:base + CW] = tags_c[1 + 7 * g:8 + 7 * g].reshape(-1)
        tr_rows[base + CW:base + 2 * CW] = \
            tags_c[1016 - 7 * g:1023 - 7 * g][::-1].reshape(-1)
    tail = NCH_E * 2 * CW
    tr_rows[tail:tail + BC] = tags_c[0]
    tr_rows[tail + BC:tail + 2 * BC] = tags_c[-1]
    tr_rows[tail + 2 * BC:] = 0.0
    tags_rows = tr_rows.reshape(1, 65536).astype(ml_dtypes.bfloat16)

    cpair = np.bincount(
        (tags_c[:-1].astype(np.int64) * T + tags_c[1:]).reshape(-1), minlength=T * T
    ).reshape(T, T).astype(np.float32)
    cs = np.bincount(tags_c[0], minlength=T).astype(np.float32)
    ce = np.bincount(tags_c[-1], minlength=T).astype(np.float32)
    return {
        "em_stack": em_stack,
        "em_init": em_init,
        "tags_rows": tags_rows,
        "iota_col": np.concatenate([np.arange(T), np.arange(T)]).reshape(128, 1).astype(np.float32),
        "ones64": np.ones((1, T), ml_dtypes.bfloat16),
        "trans2": np.concatenate([trans, trans.T], axis=0).astype(np.float32),
        "cpair": cpair,
        "cse": np.concatenate([cs, ce]).reshape(128, 1).astype(np.float32),
        "pse": np.concatenate([start, end]).reshape(128, 1).astype(np.float32),
    }


def kernel(emissions, tags, mask, transitions, start_transitions, end_transitions,
           _trace=False):
    from concourse.bass_utils import run_bass_kernel_spmd

    em = np.asarray(emissions, np.float32)
    tg = np.asarray(tags)
    tr = np.asarray(transitions, np.float32)
    st = np.asarray(start_transitions, np.float32)
    en = np.asarray(end_transitions, np.float32)
    # mask is all-ones in this problem setup; sequence lengths are full.

    in_maps = []
    for c in range(NCORES):
        sl = slice(c * BC, (c + 1) * BC)
        in_maps.append(_prep_core(
            np.ascontiguousarray(em[:, sl, :]),
            np.ascontiguousarray(tg[:, sl]).astype(np.int64),
            tr, st, en,
        ))

    nc = _get_compiled()
    res = run_bass_kernel_spmd(nc, in_maps, core_ids=list(range(NCORES)),
                               trace=_trace)
    total = 0.0
    for c in range(NCORES):
        p = res.results[c]["out_part"].reshape(-1).astype(np.float64)
        logz_sum, emit_sum, trans_dot, se_sum = p[0], p[1], p[2], p[3]
        logz_sum += BC * (S - 1) * CNORM
        total += logz_sum - (emit_sum + trans_dot + se_sum)
    out = np.float32(total / B)
    if _trace:
        return out, res
    return out
